# revision 1
# baseline (speedup 1.0000x reference)
"""Trainium2 Bass kernel for the Kalman graphical-model message-passing problem.

reference math (B=64, D=8, M=4, S=50000):
    m1 = -Qinv @ (xs - F @ x_past)            (B, D, S)
    m2 = FtQinv @ (x_fut - F @ xs)            (B, D, S)
    m3 = HtRinv @ ys_t - (HtRinv @ H) @ xs    (B, D, S)
with x_past/x_fut edge-replicated 1-sample shifts of xs along S.

Reformulated as pure (tiny matrix) x (data) products with host-precomputed
weights:
    m1 = A1 @ xs + B1 @ x_past        A1 = -Qinv,        B1 = Qinv @ F
    m2 = A2 @ xs + B2 @ x_fut         A2 = -F'QinvF,     B2 = F' @ Qinv
    m3 = A3 @ xs + sum_m C3[:, m] * ys[:, :, m]
                                      A3 = -(C3 @ H),    C3 = H' @ Rinv

Device layout (per core: 8 batches, data-parallel across 8 cores):
  A supertile covers 16 consecutive 1024-sample groups of one batch.  SBUF X
  tile [128, 1026]: partition 8g+j = (group g, state j), columns = samples
  with 1 halo column each side, so cur/past/fut are just column offsets 1/0/2
  of the same tile.  Weights are 16x block-diagonal [128, 128] lhsT matrices
  -> each output tile is a PSUM-accumulated chain of float32r matmuls at full
  column rate, processed in two 512-column halves (PSUM bank limit).  ys is
  loaded contiguously as [16, 1024*4] (partition = group, free = (t, m)
  interleaved) and contracted over m with 4 accumulating matmuls whose rhs
  access patterns stride by 4 elements.  The three outputs live in one
  [bc, D, 3, s] DRAM tensor so each supertile's store is a single DMA whose
  (state, output) dims merge into one stride-s run of 24.
"""

import os
from contextlib import ExitStack

import numpy as np

import concourse.bacc as bacc
import concourse.bass as bass
import concourse.mybir as mybir
import concourse.tile as tile
from concourse.bass_utils import run_bass_kernel_spmd

F32 = mybir.dt.float32
F32R = mybir.dt.float32r

B, D, M, S = 64, 8, 4, 50000
N_CORES = 8
BC = B // N_CORES  # batches per core
NG = 16            # sample groups packed into the 128 partitions
TCG = 1024         # samples per group per supertile
MW = 512           # matmul free-dim / PSUM bank width


def _build_nc(bc=BC, s=S):
    variant = os.environ.get("KERNEL_VARIANT", "full")  # perf bisection only
    super_sz = NG * TCG
    n_full = s // super_sz
    rem = s - n_full * super_sz
    # fp32r matmuls need an even free-dim count: round the tail width up to
    # even and overlap the previous supertile (overlapped samples are computed
    # twice with identical results).
    tc_tail = -(-rem // NG)
    tc_tail += tc_tail % 2
    tail_base = s - NG * tc_tail
    assert rem > 0 and 2 <= tc_tail <= MW and tail_base >= 1, (s, rem, tc_tail)

    nc = bacc.Bacc(trn_type="TRN2")
    xs = nc.dram_tensor("xs", [bc, D, s], F32R, kind="ExternalInput")
    ys = nc.dram_tensor("ys", [bc, s, M], F32R, kind="ExternalInput")
    w = nc.dram_tensor("w_all", [128, 9 * 128], F32R, kind="ExternalInput")
    # [b, j, o, s] layout: the store's (j, o) dims merge into one stride-s
    # run of 24, keeping the DMA access pattern at 3 dims.
    m_all = nc.dram_tensor("m_all", [bc, D, 3, s], F32, kind="ExternalOutput")

    with tile.TileContext(nc) as tc, ExitStack() as ctx:
        singles = ctx.enter_context(tc.tile_pool(name="singles", bufs=1))
        xp = ctx.enter_context(tc.tile_pool(name="xp", bufs=3))
        yp = ctx.enter_context(tc.tile_pool(name="yp", bufs=3))
        op = ctx.enter_context(tc.tile_pool(name="op", bufs=3))
        pp = ctx.enter_context(tc.tile_pool(name="pp", bufs=2, space="PSUM"))

        w_sb = singles.tile([128, 9 * 128], F32R, tag="w")
        nc.sync.dma_start(out=w_sb[:], in_=w[:, :])
        wr = w_sb[:]

        for b in range(bc):
            xoff = b * D * s
            yoff = b * s * M
            ooff = b * D * 3 * s
            for k in range(n_full + 1):
                is_tail = k == n_full
                tcw = tc_tail if is_tail else TCG
                base = tail_base if is_tail else k * super_sz
                cols = tcw + 2

                # --- load xs supertile with halo columns -------------------
                x_t = xp.tile([128, TCG + 2], F32R, tag="x")
                if k == 0:
                    # columns 1..cols-1 hold samples tcw*g .. tcw*g+tcw
                    nc.sync.dma_start(
                        out=x_t[:, 1:cols],
                        in_=bass.AP(xs, xoff, [[tcw, NG], [s, D], [1, cols - 1]]),
                    )
                    # group 0: replicate sample 0 into the past halo (aligned
                    # 32-partition copy; the halo DMA below overwrites 8..32)
                    nc.vector.tensor_copy(out=x_t[0:32, 0:1], in_=x_t[0:32, 1:2])
                    # past-halo col 0 for groups 1..15 = sample tcw*g - 1
                    nc.sync.dma_start(
                        out=x_t[D:128, 0:1],
                        in_=bass.AP(
                            xs, xoff + tcw - 1, [[tcw, NG - 1], [s, D], [1, 1]]
                        ),
                    )
                elif not is_tail:
                    nc.sync.dma_start(
                        out=x_t[:, 0:cols],
                        in_=bass.AP(
                            xs, xoff + base - 1, [[tcw, NG], [s, D], [1, cols]]
                        ),
                    )
                else:
                    # tail: columns 0..tcw valid from DRAM
                    nc.sync.dma_start(
                        out=x_t[:, 0 : tcw + 1],
                        in_=bass.AP(
                            xs, xoff + base - 1, [[tcw, NG], [s, D], [1, tcw + 1]]
                        ),
                    )
                    # last group: replicate final sample into the fut halo.
                    # DVE needs a quadrant-aligned partition base, so copy all
                    # of partitions 96..128 first; the halo DMA below then
                    # overwrites 96..120 with the true values.
                    nc.vector.tensor_copy(
                        out=x_t[96:128, tcw + 1 : tcw + 2],
                        in_=x_t[96:128, tcw : tcw + 1],
                    )
                    # fut-halo col tcw+1 for groups 0..14
                    nc.sync.dma_start(
                        out=x_t[0 : 128 - D, tcw + 1 : tcw + 2],
                        in_=bass.AP(
                            xs, xoff + base + tcw, [[tcw, NG - 1], [s, D], [1, 1]]
                        ),
                    )

                # --- load ys supertile (contiguous per group) --------------
                y_t = yp.tile([16, TCG * M], F32R, tag="y")
                nc.sync.dma_start(
                    out=y_t[:, 0 : tcw * M],
                    in_=bass.AP(ys, yoff + base * M, [[tcw * M, NG], [1, tcw * M]]),
                )
                yr = y_t[:, 0 : tcw * M].rearrange("p (t m) -> p m t", m=M)

                if variant == "loads":
                    continue
                o_t = op.tile([128, 3 * TCG], F32, tag="o", name=f"o_{b}_{k}")

                # --- matmuls + PSUM drain, in 512-column halves ------------
                for h0 in range(0, tcw, MW):
                    hw_ = min(MW, tcw - h0)
                    ps = [
                        pp.tile([128, MW], F32, tag=f"p{i}", name=f"p{i}_{b}_{k}_{h0}")
                        for i in range(3)
                    ]
                    cur = x_t[:, 1 + h0 : 1 + h0 + hw_]
                    past = x_t[:, h0 : h0 + hw_]
                    fut = x_t[:, 2 + h0 : 2 + h0 + hw_]
                    p0 = ps[0][:, 0:hw_]
                    p1 = ps[1][:, 0:hw_]
                    p2 = ps[2][:, 0:hw_]
                    nc.tensor.matmul(p0, wr[:, 0:128], cur, start=True, stop=False)
                    nc.tensor.matmul(p0, wr[:, 128:256], past, start=False, stop=True)
                    nc.tensor.matmul(p1, wr[:, 256:384], cur, start=True, stop=False)
                    nc.tensor.matmul(p1, wr[:, 384:512], fut, start=False, stop=True)
                    nc.tensor.matmul(p2, wr[:, 512:640], cur, start=True, stop=False)
                    for m in range(M):
                        c0 = (5 + m) * 128
                        nc.tensor.matmul(
                            p2,
                            wr[0:16, c0 : c0 + 128],
                            yr[:, m, h0 : h0 + hw_],
                            start=False,
                            stop=(m == M - 1),
                        )
                    if variant == "nostores":
                        continue
                    for i in range(3):
                        nc.vector.tensor_copy(
                            out=o_t[:, i * tcw + h0 : i * tcw + h0 + hw_],
                            in_=ps[i][:, 0:hw_],
                        )

                if variant == "nostores":
                    continue
                # --- one merged store DMA per supertile --------------------
                nc.scalar.dma_start(
                    out=bass.AP(
                        m_all, ooff + base, [[tcw, NG], [s, 3 * D], [1, tcw]]
                    ),
                    in_=o_t[:, 0 : 3 * tcw].rearrange("p (o t) -> p o t", o=3),
                )
    nc.finalize()
    return nc


def _build_weights(F, H, Q, R):
    """Host-side precompute (init-time work in the torch module)."""
    F64 = np.asarray(F, np.float64)
    H64 = np.asarray(H, np.float64)
    Q64 = np.asarray(Q, np.float64)
    R64 = np.asarray(R, np.float64)
    Qinv = np.linalg.inv(Q64)
    Rinv = np.linalg.inv(R64)
    A1 = -Qinv
    B1 = Qinv @ F64
    B2 = F64.T @ Qinv
    A2 = -(B2 @ F64)
    C3 = H64.T @ Rinv          # (D, M)
    A3 = -(C3 @ H64)

    w = np.zeros((128, 9 * 128), np.float32)
    eye = np.eye(NG)
    for i, A in enumerate([A1, B1, A2, B2, A3]):
        # lhsT[8g+j, 8g+i] = A[i, j]  ->  block-diag of A.T
        w[:, i * 128 : (i + 1) * 128] = np.kron(eye, A.T).astype(np.float32)
    for m in range(M):
        blk = np.zeros((NG, 128), np.float64)
        for g in range(NG):
            blk[g, D * g : D * g + D] = C3[:, m]
        w[0:NG, (5 + m) * 128 : (6 + m) * 128] = blk.astype(np.float32)
    return w


_CACHE = {}


def _get_nc(bc=BC, s=S):
    key = (bc, s)
    if key not in _CACHE:
        _CACHE[key] = _build_nc(bc, s)
    return _CACHE[key]


def run(xs, ys, F, H, Q, R, trace=False, bc=BC, s=S):
    """Shard across 8 cores, run, gather.  Returns ((m1, m2, m3), results)."""
    xs = np.ascontiguousarray(np.asarray(xs, np.float32))
    ys = np.ascontiguousarray(np.asarray(ys, np.float32))
    w_all = _build_weights(F, H, Q, R)
    nb = xs.shape[0]
    assert nb == bc * N_CORES and xs.shape[1:] == (D, s), xs.shape
    assert ys.shape == (nb, s, M), ys.shape

    nc = _get_nc(bc, s)
    in_maps = [
        {
            "xs": np.ascontiguousarray(xs[i * bc : (i + 1) * bc]),
            "ys": np.ascontiguousarray(ys[i * bc : (i + 1) * bc]),
            "w_all": w_all,
        }
        for i in range(N_CORES)
    ]
    res = run_bass_kernel_spmd(nc, in_maps, core_ids=list(range(N_CORES)), trace=trace)
    m_full = np.concatenate([r["m_all"] for r in res.results], axis=0)  # (B,D,3,s)
    outs = tuple(np.ascontiguousarray(m_full[:, :, i, :]) for i in range(3))
    return outs, res


def kernel(xs, ys, F, H, Q, R):
    trace = bool(int(os.environ.get("KERNEL_TRACE", "0")))
    outs, _ = run(xs, ys, F, H, Q, R, trace=trace)
    return outs



# revision 5
# speedup vs baseline: 2.2933x; 2.2933x over previous
"""Trainium2 Bass kernel for the Kalman graphical-model message-passing problem.

reference math (B=64, D=8, M=4, S=50000):
    m1 = -Qinv @ (xs - F @ x_past)            (B, D, S)
    m2 = FtQinv @ (x_fut - F @ xs)            (B, D, S)
    m3 = HtRinv @ ys_t - (HtRinv @ H) @ xs    (B, D, S)
with x_past/x_fut edge-replicated 1-sample shifts of xs along S.

Reformulated as pure (tiny matrix) x (data) products with host-precomputed
weights:
    m1 = A1 @ xs + B1 @ x_past        A1 = -Qinv,        B1 = Qinv @ F
    m2 = A2 @ xs + B2 @ x_fut         A2 = -F'QinvF,     B2 = F' @ Qinv
    m3 = A3 @ xs + C3 @ ys_t          A3 = -(C3 @ H),    C3 = H' @ Rinv

v2 design (fp16 I/O, memory-roofline oriented; 8-way batch data parallel):
  Host converts xs -> fp16 (B,D,S), ys -> transposed fp16 ys_t (B,M,S), and
  the output comes back as fp16, halving HBM traffic vs f32 (well within the
  2e-2 rel-err budget).  Per core (8 batches), ONE supertile per batch covers
  the whole signal: partitions = 16 groups x 8 states, each group a 3125-
  sample stripe (stride 3125) with 2 halo columns, so cur/past/fut are column
  offsets 1/0/2 of one [128, 3127] tile and DMA runs are ~6.25 KB.  The
  transposed ys loads as [64, 3125] (partition = (group, m)), letting ONE
  matmul contract all 4 observation dims -> 6 fp16 matmuls per 512-column
  PSUM half instead of 9.  PSUM drains (f32 -> fp16) are split between the
  vector and scalar engines; each message stores with a single [128, 3125]
  DMA of stride-s runs.
"""

import os
from contextlib import ExitStack

import numpy as np

import concourse.bacc as bacc
import concourse.bass as bass
import concourse.mybir as mybir
import concourse.tile as tile
from concourse.bass_utils import run_bass_kernel_spmd

F16 = mybir.dt.float16
F32 = mybir.dt.float32

B, D, M, S = 64, 8, 4, 50000
N_CORES = 8
BC = B // N_CORES  # batches per core
NG = 16            # sample groups packed into the 128 partitions
MW = 512           # matmul free-dim / PSUM bank width


def _build_nc(bc=BC, s=S):
    variant = os.environ.get("KERNEL_VARIANT", "full")  # perf bisection only
    assert s % NG == 0, s
    stride = s // NG          # samples per group (stored width)
    cols = stride + 2         # + past/fut halo columns

    nc = bacc.Bacc(trn_type="TRN2")
    xs = nc.dram_tensor("xs", [bc, D, s], F16, kind="ExternalInput")
    yt = nc.dram_tensor("yt", [bc, M, s], F16, kind="ExternalInput")
    w = nc.dram_tensor("w_all", [128, 6 * 128], F16, kind="ExternalInput")
    m_all = nc.dram_tensor("m_all", [bc, D, 3, s], F16, kind="ExternalOutput")

    with tile.TileContext(nc) as tc, ExitStack() as ctx:
        singles = ctx.enter_context(tc.tile_pool(name="singles", bufs=1))
        xp = ctx.enter_context(tc.tile_pool(name="xp", bufs=3))
        yp = ctx.enter_context(tc.tile_pool(name="yp", bufs=3))
        op = ctx.enter_context(tc.tile_pool(name="op", bufs=3))
        pp = ctx.enter_context(tc.tile_pool(name="pp", bufs=2, space="PSUM"))

        w_sb = singles.tile([128, 6 * 128], F16, tag="w")
        nc.sync.dma_start(out=w_sb[:], in_=w[:, :])
        wr = w_sb[:]

        for b in range(bc):
            xoff = b * D * s
            yoff = b * M * s
            ooff = b * D * 3 * s

            # --- xs supertile with halo columns ----------------------------
            # column c of group g = sample g*stride + c - 1
            x_t = xp.tile([128, cols], F16, tag="x")
            # group 0: cols 1..cols (samples 0..stride+1); past halo col 0
            # is replicated from col 1 below.
            nc.sync.dma_start(
                out=x_t[0:D, 1:cols],
                in_=bass.AP(xs, xoff, [[s, D], [1, cols - 1]]),
            )
            # group 15: cols 0..cols-1 (samples up to s-1); fut halo col
            # cols-1 is replicated from cols-2 below.
            nc.sync.dma_start(
                out=x_t[128 - D : 128, 0 : cols - 1],
                in_=bass.AP(
                    xs, xoff + (NG - 1) * stride - 1, [[s, D], [1, cols - 1]]
                ),
            )
            # Edge replications (32-partition aligned for DVE; the full load
            # of groups 1..14 below overwrites the clobbered partitions).
            nc.vector.tensor_copy(out=x_t[0:32, 0:1], in_=x_t[0:32, 1:2])
            nc.vector.tensor_copy(
                out=x_t[96:128, cols - 1 : cols], in_=x_t[96:128, cols - 2 : cols - 1]
            )
            # groups 1..14: full width incl. both halos
            nc.sync.dma_start(
                out=x_t[D : 128 - D, 0:cols],
                in_=bass.AP(
                    xs, xoff + stride - 1, [[stride, NG - 2], [s, D], [1, cols]]
                ),
            )

            # --- ys supertile (transposed on host: partition = (g, m)) -----
            y_t = yp.tile([64, stride], F16, tag="y")
            nc.sync.dma_start(
                out=y_t[:, :],
                in_=bass.AP(yt, yoff, [[stride, NG], [s, M], [1, stride]]),
            )

            if variant == "loads":
                continue
            o_t = op.tile([128, 3 * stride], F16, tag="o", name=f"o_{b}")

            # --- matmuls + PSUM drain, in 512-column halves ----------------
            nh = -(-stride // MW)
            for hi in range(nh):
                h0 = hi * MW
                hw_ = min(MW, stride - h0)
                ps = [
                    pp.tile([128, MW], F32, tag=f"p{i}", name=f"p{i}_{b}_{hi}")
                    for i in range(3)
                ]
                cur = x_t[:, 1 + h0 : 1 + h0 + hw_]
                past = x_t[:, h0 : h0 + hw_]
                fut = x_t[:, 2 + h0 : 2 + h0 + hw_]
                p0 = ps[0][:, 0:hw_]
                p1 = ps[1][:, 0:hw_]
                p2 = ps[2][:, 0:hw_]
                nc.tensor.matmul(p0, wr[:, 0:128], cur, start=True, stop=False)
                nc.tensor.matmul(p0, wr[:, 128:256], past, start=False, stop=True)
                nc.tensor.matmul(p1, wr[:, 256:384], cur, start=True, stop=False)
                nc.tensor.matmul(p1, wr[:, 384:512], fut, start=False, stop=True)
                nc.tensor.matmul(p2, wr[:, 512:640], cur, start=True, stop=False)
                nc.tensor.matmul(
                    p2, wr[0:64, 640:768], y_t[:, h0 : h0 + hw_], start=False, stop=True
                )
                if variant == "nostores":
                    continue
                # drains (f32 -> fp16)
                if variant == "mixdrain":
                    # balanced vector/scalar split (stores must then go on
                    # sync: Act-engine HWDGE stores + Act drains crash HW)
                    nc.vector.tensor_copy(out=o_t[:, h0 : h0 + hw_], in_=p0)
                    nc.scalar.copy(
                        out=o_t[:, stride + h0 : stride + h0 + hw_], in_=p1
                    )
                    if hi % 2 == 0:
                        nc.scalar.copy(
                            out=o_t[:, 2 * stride + h0 : 2 * stride + h0 + hw_],
                            in_=p2,
                        )
                    else:
                        nc.vector.tensor_copy(
                            out=o_t[:, 2 * stride + h0 : 2 * stride + h0 + hw_],
                            in_=p2,
                        )
                    continue
                nc.vector.tensor_copy(out=o_t[:, h0 : h0 + hw_], in_=p0)
                nc.vector.tensor_copy(
                    out=o_t[:, stride + h0 : stride + h0 + hw_], in_=p1
                )
                nc.vector.tensor_copy(
                    out=o_t[:, 2 * stride + h0 : 2 * stride + h0 + hw_], in_=p2
                )

            if variant == "nostores":
                continue
            # --- one store DMA per message ---------------------------------
            store_eng = nc.sync if variant == "mixdrain" else nc.scalar
            for o in range(3):
                store_eng.dma_start(
                    out=bass.AP(
                        m_all, ooff + o * s, [[stride, NG], [3 * s, D], [1, stride]]
                    ),
                    in_=o_t[:, o * stride : (o + 1) * stride],
                )
    nc.finalize()
    return nc


def _build_weights(F, H, Q, R):
    """Host-side precompute (init-time work in the torch module)."""
    F64 = np.asarray(F, np.float64)
    H64 = np.asarray(H, np.float64)
    Q64 = np.asarray(Q, np.float64)
    R64 = np.asarray(R, np.float64)
    Qinv = np.linalg.inv(Q64)
    Rinv = np.linalg.inv(R64)
    A1 = -Qinv
    B1 = Qinv @ F64
    B2 = F64.T @ Qinv
    A2 = -(B2 @ F64)
    C3 = H64.T @ Rinv          # (D, M)
    A3 = -(C3 @ H64)

    w = np.zeros((128, 6 * 128), np.float64)
    eye = np.eye(NG)
    for i, A in enumerate([A1, B1, A2, B2, A3]):
        # lhsT[8g+j, 8g+i] = A[i, j]  ->  block-diag of A.T
        w[:, i * 128 : (i + 1) * 128] = np.kron(eye, A.T)
    for g in range(NG):
        # lhsT[4g+m, 8g+i] = C3[i, m]
        w[4 * g : 4 * g + 4, 640 + 8 * g : 640 + 8 * g + 8] = C3.T
    return w.astype(np.float16)


_CACHE = {}


def _get_nc(bc=BC, s=S):
    key = (bc, s)
    if key not in _CACHE:
        _CACHE[key] = _build_nc(bc, s)
    return _CACHE[key]


def run(xs, ys, F, H, Q, R, trace=False, bc=BC, s=S):
    """Shard across 8 cores, run, gather.  Returns ((m1, m2, m3), results)."""
    xs16 = np.asarray(np.asarray(xs), np.float16)                    # (B, D, S)
    yt16 = np.asarray(ys, np.float32).transpose(0, 2, 1).astype(np.float16)
    w_all = _build_weights(F, H, Q, R)
    nb = xs16.shape[0]
    assert nb == bc * N_CORES and xs16.shape[1:] == (D, s), xs16.shape
    assert yt16.shape == (nb, M, s), yt16.shape

    nc = _get_nc(bc, s)
    in_maps = [
        {
            "xs": xs16[i * bc : (i + 1) * bc],
            "yt": yt16[i * bc : (i + 1) * bc],
            "w_all": w_all,
        }
        for i in range(N_CORES)
    ]
    res = run_bass_kernel_spmd(nc, in_maps, core_ids=list(range(N_CORES)), trace=trace)
    m_full = np.concatenate([r["m_all"] for r in res.results], axis=0)  # (B,D,3,s)
    outs = tuple(m_full[:, :, i, :].astype(np.float32) for i in range(3))
    return outs, res


def kernel(xs, ys, F, H, Q, R):
    trace = bool(int(os.environ.get("KERNEL_TRACE", "0")))
    outs, _ = run(xs, ys, F, H, Q, R, trace=trace)
    return outs
